# revision 63
# baseline (speedup 1.0000x reference)
"""Trainium2 Bass kernel for a dense decoder block (pre-LN MHA + FFN).

Shapes (hardcoded): B=512, T=128, D=384, H=6, DH=64, DFF=1536.
Sharding: pure data parallel -- batch split 64-per-core across 8 cores,
all weights replicated.

Per-core kernel layout (v3):
  * Token-major [T, D] tiles; LN stats are free-dim reductions.
  * Sequences processed in groups of GRP=4 (512-wide moving operands).
  * All DMA transposes are 4x batched: one DmaTranspose per 128-chunk
    moves all 4 sequences via a 3D out AP (block transpose).
  * x loads / out stores are one batched DMA per group.
  * rsqrt for LN computed as exp(-0.5*ln(var+eps)) so every ACT op
    (ln/exp/identity/copy/relu) lives in one activation-table set.
  * Engine balance: ACT does rsqrt/exp/relu/bias evictions, DVE does
    stats/mask/PSUM copies/residual adds, Pool does LN apply + softmax
    normalize + broadcast-bias adds.
  * Host-side folding: LN gains g1/g2 folded into Wq/Wk/Wv/W1; the
    attention scale folded into Wq/bq; be1@Wv@Wo folded into the
    attention-output bias; be2@W1 into the FFN1 bias.
  * Weights packed into 2 DRAM tensors (attn / ffn), biases into 1.
"""

import os
import sys
from contextlib import ExitStack

import numpy as np

for _p in ("/opt/trn_rl_repo", "/root/.axon_site/_ro/trn_rl_repo"):
    if os.path.isdir(_p) and _p not in sys.path:
        sys.path.append(_p)

import concourse.bass as bass
import concourse.tile as tile
from concourse import bacc, mybir
from concourse.masks import make_causal_mask

B, T, D, H = 512, 128, 384, 6
DH = D // H          # 64
DFF = 4 * D          # 1536
EPS = 1e-5
N_CORES = 8
BL = B // N_CORES    # 64 sequences per core
GRP = 4              # sequences per compute group (512-wide moving dims)

F32 = mybir.dt.float32
F16 = mybir.dt.float16
DC = D // 128        # 3 chunks of the model dim
FC = DFF // 128      # 12 chunks of the FFN dim
SCALE = DH ** -0.5   # 0.125

ACT = mybir.ActivationFunctionType
ALU = mybir.AluOpType

# packed attention-weight tensor column offsets (f16, [128, WATT])
_off = 0
W_OFFS = {}
for _nm, _w in (("wqA", 2 * D), ("wqB", D), ("wkA", 2 * D), ("wkB", D),
                ("wvA", 2 * D), ("wvB", D), ("woA", 2 * D), ("woB", D)):
    W_OFFS[_nm] = (_off, _w)
    _off += _w
WATT = _off
_off = 0
for _nm, _w in (("w1A", 2 * DFF), ("w1B", DFF), ("w2all", FC * D)):
    W_OFFS[_nm] = (_off, _w)
    _off += _w
WFFN = _off
# packed bias tensor (f32 flat): bq, bk, b1p, bo2, b2
B_OFFS = {"bq": (0, D), "bk": (D, D), "b1p": (2 * D, DFF),
          "bo2": (2 * D + DFF, D), "b2": (3 * D + DFF, D)}
BLEN = 4 * D + DFF


def ap3(t, offset_cols, strides_counts):
    """N-D AP into tile t at column offset; strides_counts outermost-first,
    innermost last (e.g. [[512, 3], [1, 128]])."""
    return bass.AP(tensor=t.tensor, offset=t.offset + offset_cols,
                   ap=[list(t.ap[0])] + [list(sc) for sc in strides_counts])


def build_decoder_block(tc, io, bl, grp):
    nc = tc.nc
    ctx = ExitStack()
    with ctx:
        _build(ctx, tc, nc, io, bl, grp)


def _build(ctx, tc, nc, io, bl, grp):
    x_d = io["x"]
    out_d = io["out"]
    gt = grp * T

    singles = ctx.enter_context(tc.tile_pool(name="singles", bufs=1))

    mask1 = singles.tile([128, 128], F32)
    make_causal_mask(nc, mask1, mask_val=-1e10)
    mask4 = singles.tile([128, gt], F32)
    for j in range(grp):
        nc.vector.tensor_copy(out=mask4[:, j * T:(j + 1) * T], in_=mask1)

    # --- packed weights: one tile per packed DRAM tensor, slices as views
    watt = singles.tile([128, WATT], F16, tag="watt", name="watt")
    _wqk_end = W_OFFS["wvA"][0]
    nc.sync.dma_start(out=watt[:, 0:_wqk_end], in_=io["watt"][:, 0:_wqk_end])
    nc.sync.dma_start(out=watt[:, _wqk_end:], in_=io["watt"][:, _wqk_end:])
    wffn = singles.tile([128, WFFN], F16, tag="wffn", name="wffn")

    def wv_(name):
        base = watt if name in ("wqA", "wqB", "wkA", "wkB", "wvA", "wvB",
                                "woA", "woB") else wffn
        off, width = W_OFFS[name]
        return base[:, off:off + width]

    def load_deferred_weights():
        nc.sync.dma_start(out=wffn, in_=io["wffn"])

    # --- biases from the packed flat f32 tensor
    def bias_pc(name, chunks):
        off, width = B_OFFS[name]
        t = singles.tile([128, chunks], F32, tag=f"b_{name}", name=f"b_{name}")
        nc.sync.dma_start(
            out=t, in_=io["bias"][off:off + width].rearrange("(c p) -> p c",
                                                             p=128))
        return t

    def bias_bcast(name):
        off, width = B_OFFS[name]
        t = singles.tile([128, width], F32, tag=f"b_{name}", name=f"b_{name}")
        src = io["bias"][off:off + width]
        nc.sync.dma_start(
            out=t,
            in_=bass.AP(tensor=src.tensor, offset=src.offset,
                        ap=[[0, 128]] + list(src.ap)),
        )
        return t


    bq = bias_pc("bq", DC)
    bk = bias_pc("bk", DC)
    b1 = bias_pc("b1p", FC)
    bo2 = bias_bcast("bo2")
    b2 = bias_bcast("b2")

    eps_t = singles.tile([128, 1], F32)
    nc.vector.memset(eps_t, EPS)

    psum = ctx.enter_context(tc.tile_pool(name="psum", bufs=4, space="PSUM"))
    psumw = ctx.enter_context(tc.tile_pool(name="psumw", bufs=1, space="PSUM"))
    psumf = ctx.enter_context(tc.tile_pool(name="psumf", bufs=3, space="PSUM"))
    xp = ctx.enter_context(tc.tile_pool(name="xp", bufs=4))
    stp = ctx.enter_context(tc.tile_pool(name="stp", bufs=2))
    lnp = ctx.enter_context(tc.tile_pool(name="lnp", bufs=2))
    htp = ctx.enter_context(tc.tile_pool(name="htp", bufs=2))
    qkp = ctx.enter_context(tc.tile_pool(name="qkp", bufs=2))
    vp = ctx.enter_context(tc.tile_pool(name="vp", bufs=grp + 2))
    pp = ctx.enter_context(tc.tile_pool(name="pp", bufs=4))
    pnp = ctx.enter_context(tc.tile_pool(name="pnp", bufs=2))
    ptp_ = ctx.enter_context(tc.tile_pool(name="ptp", bufs=2))
    otp = ctx.enter_context(tc.tile_pool(name="otp", bufs=2))
    o1p = ctx.enter_context(tc.tile_pool(name="o1p", bufs=2 * grp))
    ap_ = ctx.enter_context(tc.tile_pool(name="ap", bufs=2))
    obp = ctx.enter_context(tc.tile_pool(name="obp", bufs=2))

    def ln_stats(src_slices, tag):
        """Batched LN stats for 4 [128, D] f32 slices -> (mv, rs) tiles."""
        mv = stp.tile([128, 2 * grp], F32, tag=f"mv{tag}")
        for j, src in enumerate(src_slices):
            st = stp.tile([128, 6], F32, tag=f"st{tag}_{j}")
            nc.vector.bn_stats(out=st, in_=src)
            nc.vector.bn_aggr(out=mv[:, 2 * j:2 * j + 2], in_=st)
        sd = stp.tile([128, grp], F32, tag=f"sd{tag}")
        nc.scalar.activation(out=sd, in_=ap3(mv, 1, [[2, grp], [1, 1]]),
                             func=ACT.Sqrt, bias=eps_t, scale=1.0)
        rs = stp.tile([128, grp], F32, tag=f"rs{tag}")
        nc.vector.reciprocal(out=rs, in_=sd)
        return mv, rs

    def ln_apply(src_slices, mv, rs, tag):
        """LN apply into chunk-major lnG tile [128, 3*512] f16 (chunk c,
        seq j at cols c*512 + j*128). Split across Pool and DVE."""
        lnG = lnp.tile([128, DC * gt], F16, tag=f"ln{tag}", name=f"ln{tag}")
        for j, src in enumerate(src_slices):
            eng = nc.gpsimd if j % 2 == 0 else nc.vector
            eng.tensor_scalar(
                out=ap3(lnG, j * T, [[gt, DC], [1, T]]), in0=src,
                scalar1=mv[:, 2 * j:2 * j + 1], scalar2=rs[:, j:j + 1],
                op0=ALU.subtract, op1=ALU.mult)
        return lnG

    def ln_group(src_slices, tag):
        mv, rs = ln_stats(src_slices, tag)
        return ln_apply(src_slices, mv, rs, tag)

    def transpose_group(lnG, dst, split=False):
        """Batched DMA block-transpose(s): chunk c of 4 seqs per DMA when
        split (downstream can start on chunk 0), else all 12 blocks in one."""
        if split:
            for c in range(DC):
                nc.sync.dma_start_transpose(
                    out=ap3(dst, c * gt, [[T, grp], [1, T]]),
                    in_=lnG[:, c * gt:(c + 1) * gt])
        else:
            nc.sync.dma_start_transpose(
                out=ap3(dst, 0, [[T, DC * grp], [1, T]]), in_=lnG)

    def load_x(g):
        """Prefetch + LN1 stats for group g; returns (xG, xs, mv, rs)."""
        xG = xp.tile([128, grp * D], F32, tag="x", name="xG")
        b0 = g * grp
        nc.sync.dma_start(out=xG,
                          in_=x_d[b0:b0 + grp].rearrange("j t d -> t j d"))
        xs = [xG[:, j * D:(j + 1) * D] for j in range(grp)]
        mv, rs = ln_stats(xs, "1")
        return xG, xs, mv, rs

    def make_hT(nxt):
        """LN1 apply + transpose for a prefetched group -> (hT, xs)."""
        xG, xs, mv1, rs1 = nxt
        ln1 = ln_apply(xs, mv1, rs1, "1")
        hT = htp.tile([128, DC * gt], F16, tag="hT", name="hT")
        transpose_group(ln1, hT, split=True)
        return hT, xs

    n_groups = bl // grp
    prev_d = []
    cur = make_hT(load_x(0))
    nxt = load_x(1) if n_groups > 1 else None
    for g in range(n_groups):
        d_iter = iter(prev_d)

        def emit_d(n=1):
            for _ in range(n):
                for d in d_iter:
                    d()
                    break

        # ---- hT was applied+transposed during the previous group ----
        hT, xs = cur
        if g == 0:
            load_deferred_weights()
        emit_d(2)

        # ---- Q/K projections ----
        qT, kT = [], []
        for mc in range(DC):
            pq = psum.tile([128, gt], F32, tag="ps", name="pq")
            for c in range(DC):
                src = (wv_("wqA")[:, c * D + mc * 128:c * D + (mc + 1) * 128]
                       if c < 2 else wv_("wqB")[:, mc * 128:(mc + 1) * 128])
                nc.tensor.matmul(
                    pq, lhsT=src, rhs=hT[:, c * gt:(c + 1) * gt],
                    start=(c == 0), stop=(c == DC - 1))
            qs = qkp.tile([128, gt], F16, tag=f"q_{mc}", name=f"q_{mc}")
            nc.vector.tensor_scalar_add(out=qs, in0=pq,
                                        scalar1=bq[:, mc:mc + 1])
            qT.append(qs)

            pk = psum.tile([128, gt], F32, tag="ps", name="pk")
            for c in range(DC):
                src = (wv_("wkA")[:, c * D + mc * 128:c * D + (mc + 1) * 128]
                       if c < 2 else wv_("wkB")[:, mc * 128:(mc + 1) * 128])
                nc.tensor.matmul(
                    pk, lhsT=src, rhs=hT[:, c * gt:(c + 1) * gt],
                    start=(c == 0), stop=(c == DC - 1))
            ks = qkp.tile([128, gt], F16, tag=f"k_{mc}", name=f"k_{mc}")
            nc.vector.tensor_scalar_add(out=ks, in0=pk,
                                        scalar1=bk[:, mc:mc + 1])
            kT.append(ks)
            emit_d(1)

        # ---- V projection ----
        vs = []
        for j in range(grp):
            pv = psum.tile([128, D], F32, tag="ps", name="pv")
            for c in range(DC):
                nc.tensor.matmul(
                    pv, lhsT=hT[:, c * gt + j * T:c * gt + (j + 1) * T],
                    rhs=wv_("wvA")[:, c * D:(c + 1) * D] if c < 2
                    else wv_("wvB"),
                    start=(c == 0), stop=(c == DC - 1))
            vt = vp.tile([128, D], F16, tag="v", name="vt")
            nc.scalar.copy(out=vt, in_=pv)
            vs.append(vt)

        # ---- scores + unnormalized softmax, one head at a time ----
        pnG = pnp.tile([128, H * gt], F16, tag="pn", name="pnG")
        for h in range(H):
            mc, off = h // 2, (h % 2) * 64
            S = psum.tile([128, gt], F32, tag="ps", name="S")
            for j in range(grp):
                jj = slice(j * T, (j + 1) * T)
                nc.tensor.matmul(S[:, jj], lhsT=qT[mc][off:off + 64, jj],
                                 rhs=kT[mc][off:off + 64, jj])
            nc.vector.tensor_add(out=S, in0=S, in1=mask4)
            p16 = pp.tile([128, gt], F16, tag="p", name="p16")
            nc.scalar.activation(out=p16, in_=S, func=ACT.Exp)
            ls = stp.tile([128, grp], F32, tag="ls")
            nc.vector.tensor_reduce(out=ls, in_=ap3(p16, 0, [[T, grp], [1, T]]),
                                    axis=mybir.AxisListType.X, op=ALU.add)
            rl = stp.tile([128, grp], F32, tag="rl")
            nc.vector.reciprocal(out=rl, in_=ls)
            for j in range(grp):
                jj = slice(h * gt + j * T, h * gt + (j + 1) * T)
                nc.gpsimd.tensor_scalar_mul(out=pnG[:, jj],
                                            in0=p16[:, j * T:(j + 1) * T],
                                            scalar1=rl[:, j:j + 1])
            if h == 1 and g + 1 < n_groups:
                cur = make_hT(nxt)
            emit_d(1)

        # ---- batched prob transposes + attn@V, one head-pair at a time ----
        oT01 = otp.tile([128, 2 * gt], F16, tag="oT01", name="oT01")
        oT2 = otp.tile([128, gt], F16, tag="oT2", name="oT2")
        pTG = ptp_.tile([128, H * gt], F16, tag="pT", name="pTG")
        for mc in range(DC):
            for hh in (2 * mc, 2 * mc + 1):
                nc.sync.dma_start_transpose(
                    out=ap3(pTG, hh * gt, [[T, grp], [1, T]]),
                    in_=pnG[:, hh * gt:(hh + 1) * gt])
            po = psum.tile([128, gt], F32, tag="ps", name="po")
            for hh in (2 * mc, 2 * mc + 1):
                off = (hh % 2) * 64
                for j in range(grp):
                    jj = slice(j * T, (j + 1) * T)
                    nc.tensor.matmul(
                        po[off:off + 64, jj],
                        lhsT=vs[j][:, hh * DH:(hh + 1) * DH],
                        rhs=pTG[:, hh * gt + j * T:hh * gt + (j + 1) * T])
            dst = oT01[:, mc * gt:(mc + 1) * gt] if mc < 2 else oT2
            nc.vector.tensor_copy(out=dst, in_=po)
            if mc == 1 and g + 2 < n_groups:
                nxt = load_x(g + 2)
            emit_d(1)

        # ---- attention output projection + residual + LN2 ----
        o1s = []
        for j in range(grp):
            pr = psum.tile([128, D], F32, tag="ps", name="pr")
            for c in range(DC):
                nc.tensor.matmul(
                    pr,
                    lhsT=(oT01[:, c * gt + j * T:c * gt + (j + 1) * T]
                          if c < 2 else oT2[:, j * T:(j + 1) * T]),
                    rhs=wv_("woA")[:, c * D:(c + 1) * D] if c < 2
                    else wv_("woB"),
                    start=(c == 0), stop=(c == DC - 1))
            o1 = o1p.tile([128, D], F32, tag="o1", name="o1")
            nc.vector.tensor_add(out=o1, in0=pr, in1=xs[j])
            nc.gpsimd.tensor_add(out=o1, in0=o1, in1=bo2)
            o1s.append(o1)
            emit_d(1)
        ln2 = ln_group(o1s, "2")
        h2T = htp.tile([128, DC * gt], F16, tag="h2T", name="h2T")
        transpose_group(ln2, h2T)
        emit_d(len(prev_d))

        # ---- FFN of this group, deferred into the next group ----
        def make_d(g, h2T, o1s):
            a1 = ap_.tile([128, FC * gt], F16, tag="a1", name="a1")
            obG = obp.tile([128, grp * D], F32, tag="ob", name="obG")
            ops = []

            def a1_chunk(mf):
                def run():
                    pa = psumf.tile([128, gt], F32, tag="psf", name="pa")
                    for c in range(DC):
                        src = (wv_("w1A")[:, c * DFF + mf * 128:
                                          c * DFF + (mf + 1) * 128]
                               if c < 2
                               else wv_("w1B")[:, mf * 128:(mf + 1) * 128])
                        nc.tensor.matmul(
                            pa, lhsT=src, rhs=h2T[:, c * gt:(c + 1) * gt],
                            start=(c == 0), stop=(c == DC - 1))
                    nc.scalar.activation(out=a1[:, mf * gt:(mf + 1) * gt],
                                         in_=pa, func=ACT.Relu,
                                         bias=b1[:, mf:mf + 1], scale=1.0)
                return run

            def ff_j(j):
                def run():
                    pf = psumf.tile([128, D], F32, tag="psf", name="pf")
                    for k in range(FC):
                        nc.tensor.matmul(
                            pf,
                            lhsT=a1[:, k * gt + j * T:k * gt + (j + 1) * T],
                            rhs=wv_("w2all")[:, k * D:(k + 1) * D],
                            start=(k == 0), stop=(k == FC - 1))
                    dst = obG[:, j * D:(j + 1) * D]
                    nc.vector.tensor_add(out=dst, in0=pf, in1=o1s[j])
                    nc.gpsimd.tensor_add(out=dst, in0=dst, in1=b2)
                return run

            def store():
                nc.scalar.dma_start(
                    out=out_d[g * grp:(g + 1) * grp].rearrange(
                        "j t d -> t j d"),
                    in_=obG)

            for mf in range(FC):
                ops.append(a1_chunk(mf))
            for j in range(grp):
                ops.append(ff_j(j))
            ops.append(store)
            return ops

        ops = make_d(g, h2T, o1s)
        # run the first FFN1 chunks inline to fill the group-tail PE hole;
        # the last group has no successor, so run everything inline
        n_inline = len(ops) if g == n_groups - 1 else 4
        for op in ops[:n_inline]:
            op()
        prev_d = ops[n_inline:]

    for d in prev_d:
        d()


def build_nc(bl=BL, grp=GRP):
    nc = bacc.Bacc("TRN2", target_bir_lowering=False, debug=False,
                   enable_asserts=True)
    io = {}

    def inp(name, shape, dt):
        io[name] = nc.dram_tensor(name, shape, dt, kind="ExternalInput").ap()

    inp("x", [bl, T, D], F32)
    inp("watt", [128, WATT], F16)
    inp("wffn", [128, WFFN], F16)
    inp("bias", [BLEN], F32)
    io["out"] = nc.dram_tensor("out", [bl, T, D], F32,
                               kind="ExternalOutput").ap()

    with tile.TileContext(nc) as tc:
        build_decoder_block(tc, io, bl, grp)
    nc.compile()
    return nc


def _chunk_pair(w, width):
    """[D_in, width] -> A tile [128, 2*width] (chunks 0,1), B tile chunk 2."""
    a = np.concatenate([w[0:128, :], w[128:256, :]], axis=1)
    return a, w[256:384, :]


def prep_weights(Wq, Wk, Wv, Wo, bo, W1, b1, W2, b2, g1, be1, g2, be2):
    f = np.float64
    np16 = np.float16
    Wq, Wk, Wv, Wo = (np.asarray(a, f) for a in (Wq, Wk, Wv, Wo))
    W1, W2 = np.asarray(W1, f), np.asarray(W2, f)
    g1, be1, g2, be2 = (np.asarray(a, f) for a in (g1, be1, g2, be2))
    bo, b1, b2 = np.asarray(bo, f), np.asarray(b1, f), np.asarray(b2, f)

    wq = (g1[:, None] * Wq) * SCALE
    wk = g1[:, None] * Wk
    wv = g1[:, None] * Wv
    w1 = g2[:, None] * W1
    parts = {}
    for name, w in (("wq", wq), ("wk", wk), ("wv", wv), ("wo", Wo)):
        a, b = _chunk_pair(w, D)
        parts[name + "A"] = a
        parts[name + "B"] = b
    a, b = _chunk_pair(w1, DFF)
    parts["w1A"] = a
    parts["w1B"] = b
    parts["w2all"] = np.concatenate(
        [W2[k * 128:(k + 1) * 128, :] for k in range(FC)], axis=1)

    watt = np.concatenate(
        [parts[n] for n in ("wqA", "wqB", "wkA", "wkB", "wvA", "wvB",
                            "woA", "woB")], axis=1).astype(np16)
    wffn = np.concatenate(
        [parts[n] for n in ("w1A", "w1B", "w2all")], axis=1).astype(np16)

    bias = np.concatenate([
        (be1 @ Wq) * SCALE,
        be1 @ Wk,
        b1 + be2 @ W1,
        bo + (be1 @ Wv) @ Wo,
        b2,
    ]).astype(np.float32)
    return {"watt": watt, "wffn": wffn, "bias": bias}


_NC_CACHE = {}


def get_nc(bl=BL, grp=GRP):
    key = (bl, grp)
    if key not in _NC_CACHE:
        _NC_CACHE[key] = build_nc(bl, grp)
    return _NC_CACHE[key]


def kernel(**inputs):
    from concourse.bass_utils import run_bass_kernel_spmd

    x = np.asarray(inputs["x"], np.float32)
    w = prep_weights(**{k: v for k, v in inputs.items() if k != "x"})
    nc = get_nc()
    in_maps = []
    for c in range(N_CORES):
        m = dict(w)
        m["x"] = np.ascontiguousarray(x[c * BL:(c + 1) * BL])
        in_maps.append(m)
    res = run_bass_kernel_spmd(nc, in_maps, list(range(N_CORES)))
    return np.concatenate([r["out"] for r in res.results], axis=0)
